# revision 14
# baseline (speedup 1.0000x reference)
"""MHSA Trainium2 kernel: 8-core batch(2) x head-quad(4) sharding.

Reference: x[2,2048,1024] @ w_qkv.T -> per-head attention -> @ w_out.T + b.
Core c = 4*g + j handles batch g, heads 4j..4j+3. Host sums the 4 partials
per batch and adds the bias. All matmuls bf16, accumulation fp32.

Structure (v2):
- Stage 1: Q^T/K^T head-pair tiles [128dims, 2048tok] (PE, full 128x128),
  V natural [tok, head, d+ones] tiles. PSUM evacuation on DVE only.
- Stage 2 per (head, t-half): pass A: S^T = K Q^T scores into [128,1024]
  2-bank PSUM tiles, ONE exp per tile on ACT ([s=128, t=1024] reads), es
  resident in SBUF for the whole (head, t-half).
- Pass B: flipped AV: out[t,d+1] = es_chunk^T @ V_aug with full 128
  contraction AND 128 output rows (vs 65 in the unflipped orientation);
  the ones column gives the softmax denominator per PARTITION, so
  normalization is a DVE reciprocal + tensor_scalar (no broadcast at all).
- O [t, i] -> O^T [i, t] via DMA-engine xbar transpose (no compute engine).
- Stage 4: out-proj partial on PE, DVE evacuation, SP-issued output DMA.
ACT runs ONLY the 128 exp instructions; Pool only tiny memsets; all DMAs
issue from SP (HWDGE) so Pool never pays SWDGE descriptor generation.
"""
import numpy as np
import ml_dtypes

HEADS = 16
HEAD_DIM = 64
TOKEN_DIM = 1024
INNER = HEADS * HEAD_DIM
B = 2
N = 2048
HPC = 4            # heads per core
GROUPS = 2         # batches
CORES = 8

_cache = {}


def _build():
    import concourse.bass as bass
    import concourse.mybir as mybir
    from concourse.tile import TileContext

    F32 = mybir.dt.float32
    BF16 = mybir.dt.bfloat16
    AF = mybir.ActivationFunctionType

    from concourse.vector_clock import ScopedClock

    class TC(TileContext):
        # this walrus build allows only ONE sync wait per instruction; split
        # the kernel-tail drain's waits into standalone wait_ge instructions
        def _drain_and_barrier(self, tick_clock, wait_clock):
            any_sem = next(iter(self.sems.allocated().values()))
            tmp = self.nc.sync.wait_ge(any_sem, 0)
            wait_clock.add_sem_waits(
                tmp.ins, ScopedClock({None: tick_clock.global_clock})
            )
            waits = list(tmp.ins.sync_info.on_wait)
            try:
                tmp.ins.sync_info.on_wait.clear()
            except Exception:
                import concourse.mybir as _mybir
                tmp.ins.sync_info = _mybir.SyncInfo(
                    on_wait=[], on_update=list(tmp.ins.sync_info.on_update)
                )
            sem_by_name = {}
            for k, h in self.sems.allocated().items():
                sem_by_name[getattr(h, "name", None)] = h
                sem_by_name[str(k)] = h
            for w in waits:
                h = sem_by_name.get(getattr(w, "ant_name", None))
                if h is not None:
                    self.nc.sync.wait_ge(h, w.wait_value)
            self.nc.sync.drain()
            self.nc.all_engine_barrier()
            assert self.sems is not None
            popped = self.nc._tile_sem_poison_stack.pop()
            assert popped is self._sem_poison
            self.nc.clear_and_free_semaphores(list(self.sems.allocated().values()))
            self.nc.all_engine_barrier()

    nc = bass.Bass()
    # per-core inputs (host pre-transposed / pre-cast to bf16)
    xT = nc.declare_dram_parameter("xT", [TOKEN_DIM, N], BF16, isOutput=False)
    wqkvT = nc.declare_dram_parameter("wqkvT", [TOKEN_DIM, 3 * HPC * HEAD_DIM], BF16, isOutput=False)
    woT = nc.declare_dram_parameter("woT", [HPC * HEAD_DIM, TOKEN_DIM], BF16, isOutput=False)
    part = nc.declare_dram_parameter("part", [N, TOKEN_DIM], F32, isOutput=True)

    NT512 = N // 512      # 4
    NT128 = N // 128      # 16
    NTC = 1024 // 128     # 8 t-chunks per t-half
    CCH = TOKEN_DIM // 128  # 8 contraction chunks

    with TC(nc) as tc:
        with (
            tc.tile_pool(name="wsb", bufs=1) as wsb,
            tc.tile_pool(name="qksb", bufs=1) as qksb,
            tc.tile_pool(name="sb", bufs=3) as sb,
            tc.tile_pool(name="ps", bufs=2, space="PSUM") as ps,
        ):
            # ---- load inputs (SP-issued HWDGE DMAs) ----
            # order: q/k weights, x, v weights, w_out (by first use)
            xT_sb = wsb.tile([128, CCH, N], BF16)      # [c-part, c-chunk, t]
            xT_r = xT[:].rearrange("(c p) t -> p c t", p=128)
            wq_sb = wsb.tile([128, CCH, 3 * HPC * HEAD_DIM], BF16)
            wq_r = wqkvT[:].rearrange("(c p) r -> p c r", p=128)
            VR = HPC * HEAD_DIM  # 256
            nc.sync.dma_start(wq_sb[:, :, :2 * VR], wq_r[:, :, :2 * VR])
            for cc in range(CCH):
                nc.sync.dma_start(xT_sb[:, cc], xT_r[:, cc])
            nc.sync.dma_start(wq_sb[:, :, 2 * VR:], wq_r[:, :, 2 * VR:])
            wo_sb = wsb.tile([128, 2, TOKEN_DIM], BF16)  # [i-part, i-chunk, o]
            nc.sync.dma_start(wo_sb[:], woT[:].rearrange("(c p) o -> p c o", p=128))

            # ---- stage 1a: QT/KT head-pair tiles [128 dims, N tok] ----
            qk_tiles = {
                m: qksb.tile([128, N], BF16, name=f"qk_{m}", tag=f"qk_{m}")
                for m in range(4)
            }

            def emit_qk_chunk(m, t4):
                qk_t = qk_tiles[m]
                qkps = ps.tile([128, 512], F32, tag="gps", bufs=2,
                               name=f"qkps_{m}_{t4}")
                for cc in range(CCH):
                    nc.tensor.matmul(
                        qkps[:],
                        wq_sb[:, cc, m * 128:(m + 1) * 128],
                        xT_sb[:, cc, t4 * 512:(t4 + 1) * 512],
                        start=(cc == 0), stop=(cc == CCH - 1),
                    )
                nc.vector.tensor_copy(qk_t[:, t4 * 512:(t4 + 1) * 512], qkps[:])

            # minimal prefix for head 0, t2=0: k01 cols 0:512, q01 cols 0:1024
            # (remaining chunks are emitted as pass-A fillers)
            emit_qk_chunk(2, 0)
            emit_qk_chunk(0, 0)
            emit_qk_chunk(0, 1)

            # ---- stage 1b: V natural [tok, h, d+1] with ones column ----
            # (emitted lazily, interleaved into head-0 pass A for PE overlap)
            v_tiles = [None] * NT128

            def emit_v(t16):
                v_t = qksb.tile([128, HPC, HEAD_DIM + 1], BF16,
                                name=f"v_{t16}", tag=f"v_{t16}")
                v_tiles[t16] = v_t
                vps = ps.tile([128, 512], F32, tag="gps", bufs=2,
                              name=f"vps_{t16}")
                for cc in range(CCH):
                    nc.tensor.matmul(
                        vps[:, :VR],
                        xT_sb[:, cc, t16 * 128:(t16 + 1) * 128],
                        wq_sb[:, cc, 2 * VR:3 * VR],
                        start=(cc == 0), stop=(cc == CCH - 1),
                    )
                nc.vector.tensor_copy(
                    v_t[:, :, :HEAD_DIM],
                    vps[:, :VR].rearrange("p (h d) -> p h d", h=HPC),
                )
                nc.gpsimd.memset(v_t[:, :, HEAD_DIM:], 1.0)

            # ---- stage 2+3: attention per (head, t-half) ----
            # o_norm packs head pairs: [tok-part, tc, i(2x64)] awaiting transpose
            o_norm = [qksb.tile([128, NT128, 128], BF16, name=f"onrm_{hp}",
                                tag=f"onrm_{hp}") for hp in range(2)]
            # O^T tiles per pair [i(2x64), tok] for the out-projection
            o_all = [qksb.tile([128, N], BF16, name=f"o_{hp}", tag=f"o_{hp}")
                     for hp in range(2)]

            def passA_stepper(h, t2, filler=None):
                """scores + exp for one (h, t2); returns (es tile, step fn).
                `filler(s16)` emits extra PE work into each ACT-bound step."""
                hp, ho = h // 2, (h % 2) * 64
                kt = qk_tiles[2 + hp]
                qt = qk_tiles[hp]
                es_all = sb.tile([128, NT128, 1024], BF16, tag="es", bufs=3,
                                 name=f"es_{h}_{t2}")

                def step(s16):
                    sps = ps.tile([128, 1024], F32, tag="sps", bufs=2,
                                  name=f"sps_{h}_{t2}_{s16}")
                    for half in range(2):
                        nc.tensor.matmul(
                            sps[:, half * 512:(half + 1) * 512],
                            kt[ho:ho + 64, s16 * 128:(s16 + 1) * 128],
                            qt[ho:ho + 64, t2 * 1024 + half * 512:
                               t2 * 1024 + (half + 1) * 512],
                            start=True, stop=True,
                        )
                    nc.scalar.activation(es_all[:, s16, :], sps[:], AF.Exp)
                    if filler is not None:
                        filler(s16)

                return es_all, step

            def emit_passA(h, t2, filler=None):
                es_all, step = passA_stepper(h, t2, filler)
                for s16 in range(NT128):
                    step(s16)
                return es_all

            def emit_passB(h, t2, es_all, post_tc=None, astep=None):
                """flipped AV + per-partition normalization. `post_tc(tc)`
                emits tail work (transpose/out-proj) right after chunk tc;
                `astep` interleaves two next-pass-A score/exp steps per
                chunk so ACT never drains between phases."""
                hp, ho = h // 2, (h % 2) * 64
                for tc in range(NTC):
                    av = ps.tile([128, HEAD_DIM + 1], F32, tag="av", bufs=2,
                                 name=f"av_{h}_{t2}_{tc}")
                    for s16 in range(NT128):
                        nc.tensor.matmul(
                            av[:],
                            es_all[:, s16, tc * 128:(tc + 1) * 128],
                            v_tiles[s16][:, h, :],
                            start=(s16 == 0), stop=(s16 == NT128 - 1),
                        )
                    rec = sb.tile([128, 1], F32, tag="rec", bufs=3,
                                  name=f"rec_{h}_{t2}_{tc}")
                    nc.vector.reciprocal(rec[:], av[:, HEAD_DIM:])
                    nc.vector.tensor_scalar(
                        o_norm[hp][:, t2 * NTC + tc, ho:ho + 64],
                        av[:, :HEAD_DIM], rec[:], None,
                        op0=mybir.AluOpType.mult,
                    )
                    if post_tc is not None:
                        post_tc(t2 * NTC + tc)
                    if astep is not None:
                        astep(2 * tc)
                        astep(2 * tc + 1)

            def emit_transpose(hp, tc):
                nc.sync.dma_start_transpose(
                    o_all[hp][:, tc * 128:(tc + 1) * 128],
                    o_norm[hp][:, tc, :],
                )

            def emit_outproj(t16):
                out_sb = sb.tile([128, TOKEN_DIM], F32, tag="outsb", bufs=3,
                                 name=f"outsb_{t16}")
                for o2 in range(2):
                    pps = ps.tile([128, 512], F32, tag="gps", bufs=2,
                                  name=f"pps_{t16}_{o2}")
                    for hp in range(2):
                        nc.tensor.matmul(
                            pps[:],
                            o_all[hp][:, t16 * 128:(t16 + 1) * 128],
                            wo_sb[:, hp, o2 * 512:(o2 + 1) * 512],
                            start=(hp == 0), stop=(hp == 1),
                        )
                    nc.vector.tensor_copy(out_sb[:, o2 * 512:(o2 + 1) * 512],
                                          pps[:])
                nc.sync.dma_start(
                    part[t16 * 128:(t16 + 1) * 128, :],
                    out_sb[:],
                )

            # h0 pass A absorbs the remaining QK-pair-0 chunks (k before its
            # s16 consumers) and the V-tile production in its ACT-bound steps
            a00_fill = [lambda: emit_qk_chunk(2, 1), lambda: emit_qk_chunk(2, 2),
                        lambda: emit_qk_chunk(2, 3), lambda: emit_qk_chunk(0, 2),
                        lambda: emit_qk_chunk(0, 3)] + \
                       [(lambda i=i: emit_v(i)) for i in range(11)]
            es00 = emit_passA(0, 0, filler=lambda s16: a00_fill[s16]())
            es01 = emit_passA(0, 1,
                              filler=lambda s16: emit_v(11 + s16) if s16 < 5 else None)

            # software pipeline: each pass B interleaves the next pass A's
            # score/exp steps so ACT runs gap-free across phase boundaries
            qk23 = [(m, t4) for t4 in range(NT512) for m in (1, 3)]
            fillers = {
                (1, 0): lambda s16: emit_qk_chunk(*qk23[s16 // 2])
                if s16 % 2 == 0 else None,
            }
            posts = {
                (1, 0): lambda tc: emit_transpose(0, tc),
                (1, 1): lambda tc: emit_transpose(0, tc),
            }

            def h3_post(tc):
                emit_transpose(1, tc)
                emit_outproj(tc)

            posts[(3, 0)] = h3_post
            posts[(3, 1)] = h3_post

            seq = [(h, t2) for h in range(4) for t2 in range(2)]
            es_by = {(0, 0): es00, (0, 1): es01}
            for i, b in enumerate(seq):
                a = seq[i + 2] if i + 2 < len(seq) else None
                astep = None
                if a is not None:
                    es_by[a], astep = passA_stepper(a[0], a[1], fillers.get(a))
                emit_passB(b[0], b[1], es_by.pop(b), post_tc=posts.get(b),
                           astep=astep)
    # this walrus build allows only ONE sync wait per instruction: hoist
    # extra waits onto standalone event-semaphore carriers on the same engine
    nsplit = 0
    for bb in nc.m.functions[0].blocks:
        new_insts = []
        for ins in bb.instructions:
            si = getattr(ins, "sync_info", None)
            if si is not None and len(si.on_wait) > 1:
                waits = list(si.on_wait)
                for w in waits[:-1]:
                    nsplit += 1
                    ev = mybir.InstEventSemaphore(
                        name=f"I-wsplit-{nsplit}", ins=[], outs=[],
                        engine=ins.engine,
                        sync_info=mybir.SyncInfo(on_wait=[w], on_update=[]),
                    )
                    new_insts.append(ev)
                try:
                    si.on_wait.clear()
                    si.on_wait.append(waits[-1])
                except Exception:
                    ins.sync_info = mybir.SyncInfo(
                        on_wait=[waits[-1]], on_update=list(si.on_update)
                    )
            new_insts.append(ins)
        bb.instructions = new_insts
    return nc


def kernel(x, w_qkv, w_out, b_out):
    from concourse.bass_utils import run_bass_kernel_spmd

    if "nc" not in _cache:
        _cache["nc"] = _build()
    nc = _cache["nc"]

    bf = ml_dtypes.bfloat16
    scale = HEAD_DIM ** -0.5
    x = np.asarray(x)
    w_qkv = np.asarray(w_qkv)
    w_out = np.asarray(w_out)
    b_out = np.asarray(b_out)

    in_maps = []
    for c in range(CORES):
        g, j = c // 4, c % 4
        hsl = slice(j * HPC * HEAD_DIM, (j + 1) * HPC * HEAD_DIM)
        wq = w_qkv[0 * INNER:1 * INNER][hsl] * scale   # fold softmax scale into Q
        wk = w_qkv[1 * INNER:2 * INNER][hsl]
        wv = w_qkv[2 * INNER:3 * INNER][hsl]
        wqkvT = np.concatenate([wq, wk, wv], 0).T.astype(bf)  # [1024, 768]
        woT = w_out[:, hsl].T.astype(bf)                      # [256, 1024]
        in_maps.append({
            "xT": np.ascontiguousarray(x[g].T).astype(bf),
            "wqkvT": np.ascontiguousarray(wqkvT),
            "woT": np.ascontiguousarray(woT),
        })

    res = run_bass_kernel_spmd(nc, in_maps, list(range(CORES)))
    _cache["last_res"] = res
    out = np.empty((B, N, TOKEN_DIM), dtype=np.float32)
    for g in range(GROUPS):
        acc = res.results[4 * g]["part"].astype(np.float32).copy()
        for j in range(1, 4):
            acc += res.results[4 * g + j]["part"]
        out[g] = acc + b_out[None, :]
    return out
